# revision 1
# baseline (speedup 1.0000x reference)
"""Trainium2 Bass kernel for GNN multi-head cross-attention message passing.

Math (see reference): per edge e: score[e,h,g] = qh[A[e],h,:] . kh[B[e],g,:]
segment-MEAN over destination A -> softmax over g -> att @ vh -> Wc projection.

Algebraic structure (same as v1):
  sums[n,h,g] = qh[n,h,:] . S[n,g,:],  S = (segment_sum of raw k rows) @ Wk^T
so the [E,H,H] tensor is never materialized and k is projected after
aggregation.

v2 optimizations (all validated against the TimelineSim cost model):
 - everything on the PE runs in bf16/fp8 (fp32 matmuls cost 4x cycles/row)
 - edge k-rows stream in fp8e4 (numerically validated: rel err 6e-3 vs 2e-2
   budget); one-hot scatter matrices are built on the HOST and streamed as
   fp8 too, freeing the DVE of ~75us of is_equal work per core
 - edges are sorted by destination, so each 128-edge tile's one-hot only
   needs a narrow destination WINDOW (~16 cols, host-computed); the U
   accumulation matmuls use W-wide moving operands -> ~6x less PE time
 - U is accumulated TRANSPOSED (U^T[ch, dest]) directly in PSUM, which
   kills the per-block PE transposes + copies of v1; the PSUM region is
   reset by a full-width zero matmul and every window accumulates with
   start=False (per-region start=True flags proved unreliable on HW)
 - Wv columns are permuted host-side so vh lands in (d,g) layout: every big
   DVE multiply has packed last dims on all operands -> 2x DVE mode
 - the score d-reduction runs ENTIRELY on the PE as 32 accumulating
   identity matmuls into PSUM (fp32), and exp reads the sums straight from
   PSUM -- no DVE tree, no copies; the V-phase g-reduction is a bf16
   halving tree split between DVE and GPSIMD, with the 1/den softmax
   normalization as a GPSIMD multiply after the g-sum (gpsimd divide does
   not compile; reciprocal on DVE + mult on GPSIMD does)
 - the final projection computes out^T = Wc @ ov^T so its bias is one tiny
   matmul row and the result DMAs out at full line width; the host
   un-transposes (free)
"""

import numpy as np
import ml_dtypes

import concourse.bass as bass
import concourse.mybir as mybir
import concourse.tile as tile
from concourse.bass_utils import run_bass_kernel_spmd
from concourse.masks import make_identity

# ---------------------------------------------------------------- constants
NCORES = 8
N_NODES = 50000
EMB = 256
H = 8
D = 32
P = 128

NPC = N_NODES // NCORES          # 6250 nodes per core
NB = (NPC + P - 1) // P          # 49 blocks of 128 nodes per core
NPC_PAD = NB * P                 # 6272

FP = mybir.dt.float32
BF = mybir.dt.bfloat16
F8 = mybir.dt.float8e4

NP_BF = ml_dtypes.bfloat16
NP_F8 = ml_dtypes.float8_e4m3fn


# ------------------------------------------------------- sync-wait splitting
# The staged walrus accepts only ONE sync-wait command per instruction.
# Tile attaches several waits to some instructions.  Post-pass: hoist all but
# one wait of each over-limit instruction onto same-engine Drain carriers
# placed immediately before it (engine streams execute in block order, so
# "all waits hold before the instruction runs" is preserved).
_WS_COUNTER = [0]


def _split_sync_waits(nc, maxw=1):
    for f in nc.m.functions:
        for blk in f.blocks:
            insts = blk.instructions
            out = []
            changed = False
            for ins in insts:
                si = ins.sync_info
                if si is not None and len(si.on_wait) > maxw:
                    waits = list(si.on_wait)
                    k = len(waits) - maxw
                    for i in range(0, k, maxw):
                        _WS_COUNTER[0] += 1
                        d = mybir.InstDrain(
                            name=f"I-wsplit-{_WS_COUNTER[0]}", ins=[], outs=[]
                        )
                        d.engine = ins.engine
                        d.sync_info = mybir.SyncInfo(
                            on_wait=waits[i : i + maxw], on_update=[]
                        )
                        out.append(d)
                    si.on_wait = waits[k:]
                    changed = True
                out.append(ins)
            if changed:
                blk.instructions = out


# ------------------------------------------------------------- device kernel
def build_nc(tiles_per_block, windows, split_waits=True):
    """Build the SPMD Bass module.

    tiles_per_block[b] = edge tiles in block b (same across cores).
    windows[b] = list of (doff, W) per tile: the destination window the
    tile's one-hot covers (same across cores; host guarantees coverage).
    """
    SW = [int(sum(w for _, w in wb)) for wb in windows]   # one-hot cols/block

    nc = bass.Bass("TRN2", target_bir_lowering=False, debug=False,
                   num_devices=NCORES)

    # per-core inputs (one DMA per block per stream: 650ns fixed cost/DMA)
    qv_d = nc.dram_tensor("qv", [P, NB, 4, P], BF, kind="ExternalInput")
    KOW = [int(tiles_per_block[b]) * EMB + SW[b] for b in range(NB)]
    ko_d = nc.dram_tensor("ko", [P, sum(KOW)], F8, kind="ExternalInput")
    WqT = nc.dram_tensor("WqT", [EMB, EMB], BF, kind="ExternalInput")
    WkT = nc.dram_tensor("WkT", [EMB, EMB], BF, kind="ExternalInput")
    WvT = nc.dram_tensor("WvT", [EMB, EMB], BF, kind="ExternalInput")  # perm
    WcT = nc.dram_tensor("WcT", [EMB, EMB], BF, kind="ExternalInput")
    bq = nc.dram_tensor("bq", [1, EMB], BF, kind="ExternalInput")
    bk = nc.dram_tensor("bk", [1, EMB], BF, kind="ExternalInput")
    bv = nc.dram_tensor("bv", [1, EMB], BF, kind="ExternalInput")  # perm
    bc = nc.dram_tensor("bc", [1, EMB], BF, kind="ExternalInput")
    cnt_d = nc.dram_tensor("cnt", [1, NPC_PAD], BF, kind="ExternalInput")
    invc_d = nc.dram_tensor("invc", [P, NB], FP, kind="ExternalInput")

    outT_d = nc.dram_tensor("outT", [P, NB, 2, P], FP, kind="ExternalOutput")

    with tile.TileContext(nc) as tc:
        with (
            tc.tile_pool(name="const", bufs=1) as cp,
            tc.tile_pool(name="work", bufs=7) as wp,
            tc.tile_pool(name="kep", bufs=4) as kp,
            tc.tile_pool(name="ps_qv", bufs=1, space="PSUM") as pqv,
            tc.tile_pool(name="ps_u", bufs=1, space="PSUM") as pu,
            tc.tile_pool(name="ps_d", bufs=2, space="PSUM") as pd_,
            tc.tile_pool(name="ps_acc", bufs=1, space="PSUM") as pacc,
            tc.tile_pool(name="ps_sc", bufs=1, space="PSUM") as psc,
            tc.tile_pool(name="ps_t", bufs=1, space="PSUM") as pt,
            tc.tile_pool(name="ps_o", bufs=1, space="PSUM") as po,
            tc.tile_pool(name="wl", bufs=8) as wl,
        ):
            # ---------------- constants
            ident = cp.tile([P, P], BF)
            make_identity(nc, ident[:])
            ones1 = cp.tile([1, P], BF)
            nc.vector.memset(ones1[:], 1.0)
            zf8 = cp.tile([P, P], F8)
            nc.vector.memset(zf8[:], 0.0)

            wtiles = {}
            for nm, t in (("Wq", WqT), ("Wk", WkT), ("Wv", WvT), ("Wc", WcT)):
                a = cp.tile([P, EMB], BF, tag=f"{nm}a")
                b = cp.tile([P, EMB], BF, tag=f"{nm}b")
                nc.sync.dma_start(a[:], t[0:P, :])
                nc.sync.dma_start(b[:], t[P:EMB, :])
                wtiles[nm] = (a, b)
            btiles = {}
            for nm, t in (("bq", bq), ("bk", bk), ("bv", bv), ("bc", bc)):
                s = cp.tile([1, EMB], BF, tag=nm)
                nc.sync.dma_start(s[:], t[:])
                btiles[nm] = s

            cnt_sb = cp.tile([1, NPC_PAD], BF)
            nc.sync.dma_start(cnt_sb[:], cnt_d[:])
            invc_sb = cp.tile([P, NB], FP)
            nc.sync.dma_start(invc_sb[:], invc_d[:])

            wqa, wqb = wtiles["Wq"]
            wka, wkb = wtiles["Wk"]
            wva, wvb = wtiles["Wv"]
            wca, wcb = wtiles["Wc"]

            # ---------------- software-pipelined main loop
            # Stages (iteration offsets) chosen so every engine's in-order
            # program is a round-robin of ready work; cross-engine deps either
            # span a full iteration or land late enough in both streams that
            # the consumer engine has already drained its other work.
            #   S0(b)@b    SP   qv4 + ko DMAs
            #   S1(b)@b+1  PE   windowed U^T accumulation (zero-matmul
            #                   reset, then all windows accumulate)
            #   S2(b)@b+2  ACT  uT/qv/s copies, PE qv/S projections
            #   S3(b)@b+3  DVE  prod, sr1, sr2
            #   S3b(b)@b+4 Pool sr3, sr4, sc; ACT exp
            #   S4(b)@b+5  DVE  den, recip, att, p2, vr1
            #   S4b(b)@b+6 Pool vr2, ov
            #   S5(b)@b+7  PE   transposes + out-proj; ACT copies; SP out DMA
            st = {}
            ko_off = [0]
            for b in range(NB):
                ko_off.append(ko_off[-1] + KOW[b])
            KOWMAX = max(KOW)

            def S0(b):
                # qv4[p, b, j, n]: j = (q ch-lo, v ch-lo, q ch-hi, v ch-hi)
                qv4 = wl.tile([P, 4, P], BF, tag="qv4")
                nc.sync.dma_start(qv4[:], qv_d[:, b, :, :])
                # fused k-rows + one-hot stream for this block
                ko = kp.tile([P, KOWMAX], F8, tag="ko")
                nc.sync.dma_start(ko[:, 0:KOW[b]], ko_d[:, ko_off[b]:ko_off[b + 1]])
                st["qv4", b] = qv4
                st["ko", b] = ko

            def S1(b):
                ko = st.pop(("ko", b))
                T = int(tiles_per_block[b])
                ke = ko[:, 0:T * EMB].rearrange("p (t c) -> p t c", t=T)
                oh = ko[:, T * EMB:T * EMB + SW[b]]
                ps_uT = pu.tile([P, 2 * P], FP, space="PSUM", tag="uT")
                # reset both halves with a zero matmul (start=True over the
                # full width), then accumulate every window with start=False:
                # per-region start flags proved unreliable on HW
                for hlf in range(2):
                    nc.tensor.matmul(
                        out=ps_uT[:, hlf * P:(hlf + 1) * P],
                        lhsT=zf8[:], rhs=zf8[:],
                        start=True, stop=False, skip_group_check=True,
                    )
                wo = 0
                for t in range(T):
                    doff, W = windows[b][t]
                    last = t == T - 1
                    if W > 0:
                        for hlf in range(2):
                            nc.tensor.matmul(
                                out=ps_uT[:, hlf * P + doff:hlf * P + doff + W],
                                lhsT=ke[:, t, hlf * P:(hlf + 1) * P],
                                rhs=oh[:, wo:wo + W],
                                start=False, stop=last,
                                skip_group_check=True,
                            )
                    wo += W
                st["ps_uT", b] = ps_uT

            def S2(b):
                qv4 = st.pop(("qv4", b))
                ps_uT = st.pop(("ps_uT", b))
                uT_sb = wp.tile([P, 2 * P], BF, tag="uT_sb")
                nc.scalar.copy(uT_sb[:], ps_uT[:])
                ps_qv = pqv.tile([P, 2 * EMB], FP, space="PSUM", tag="qv")
                nc.tensor.matmul(out=ps_qv[:, 0:EMB], lhsT=qv4[:, 0, :], rhs=wqa[:], start=True, stop=False)
                nc.tensor.matmul(out=ps_qv[:, 0:EMB], lhsT=qv4[:, 2, :], rhs=wqb[:], start=False, stop=False)
                nc.tensor.matmul(out=ps_qv[:, 0:EMB], lhsT=ones1[:], rhs=btiles["bq"][:], start=False, stop=True)
                nc.tensor.matmul(out=ps_qv[:, EMB:2 * EMB], lhsT=qv4[:, 1, :], rhs=wva[:], start=True, stop=False)
                nc.tensor.matmul(out=ps_qv[:, EMB:2 * EMB], lhsT=qv4[:, 3, :], rhs=wvb[:], start=False, stop=False)
                nc.tensor.matmul(out=ps_qv[:, EMB:2 * EMB], lhsT=ones1[:], rhs=btiles["bv"][:], start=False, stop=True)
                ps_s = pacc.tile([P, EMB], FP, space="PSUM", tag="s")
                nc.tensor.matmul(out=ps_s[:], lhsT=uT_sb[:, 0:P], rhs=wka[:], start=True, stop=False)
                nc.tensor.matmul(out=ps_s[:], lhsT=uT_sb[:, P:2 * P], rhs=wkb[:], start=False, stop=False)
                nc.tensor.matmul(out=ps_s[:], lhsT=cnt_sb[:, b * P:(b + 1) * P],
                                 rhs=btiles["bk"][:], start=False, stop=True)
                qv_sb = wl.tile([P, 2 * EMB], BF, tag="qv_sb")
                nc.scalar.copy(qv_sb[:], ps_qv[:])
                s_sb = wp.tile([P, EMB], BF, tag="s_sb")
                nc.scalar.copy(s_sb[:], ps_s[:])
                st["qv_sb", b] = qv_sb
                st["s_sb", b] = s_sb

            def S3(b):
                qv_sb = st[("qv_sb", b)]
                s_sb = st.pop(("s_sb", b))
                qh_sb = qv_sb[:, 0:EMB]          # [n, (h,d)]
                # score: sums[n,h,g] = sum_d qh[n,h,d] * S[n,g,d]
                # products on DVE; the first halving add runs on the PE as an
                # identity-matmul accumulation (rhs limit 512 -> 4 matmuls)
                prod = wp.tile([P, H, H, D], BF, tag="prod")
                nc.vector.tensor_tensor(
                    out=prod[:],
                    in0=qh_sb.rearrange("p (h d) -> p h d", h=H).unsqueeze(2).to_broadcast([P, H, H, D]),
                    in1=s_sb[:].rearrange("p (g d) -> p g d", g=H).unsqueeze(1).to_broadcast([P, H, H, D]),
                    op=mybir.AluOpType.mult,
                )
                st["prod", b] = prod

            def S3p(b):
                prod = st.pop(("prod", b))
                # entire d-reduction as identity-matmul accumulation on the
                # PE: sums[n,(h,g)] = sum_d prod[n,(h,g),d], fp32 in PSUM
                ps_sc = psc.tile([P, H * H], FP, space="PSUM", tag="sc")
                for dd in range(D):
                    nc.tensor.matmul(out=ps_sc[:], lhsT=ident[:],
                                     rhs=prod[:, :, :, dd],
                                     start=(dd == 0), stop=(dd == D - 1))
                st["ps_sc", b] = ps_sc

            def S3f(b):
                ps_sc = st.pop(("ps_sc", b))
                ex = wp.tile([P, H, H], BF, tag="ex")
                nc.scalar.activation(out=ex[:],
                                     in_=ps_sc[:].rearrange("p (h g) -> p h g", h=H),
                                     func=mybir.ActivationFunctionType.Exp,
                                     scale=invc_sb[:, b:b + 1])
                st["ex", b] = ex

            def S4(b):
                qv_sb = st.pop(("qv_sb", b))
                ex = st[("ex", b)]
                vh_sb = qv_sb[:, EMB:2 * EMB]    # [n, (d,g)]  (Wv col-perm)
                # V phase, unnormalized: ovr[n,h,d] = sum_g ex[n,h,g]*vh[n,g,d]
                p2 = wp.tile([P, H, D, H], BF, tag="p2")
                nc.vector.tensor_tensor(
                    out=p2[:],
                    in0=ex[:].unsqueeze(2).to_broadcast([P, H, D, H]),
                    in1=vh_sb.rearrange("p (d g) -> p d g", d=D).unsqueeze(1).to_broadcast([P, H, D, H]),
                    op=mybir.AluOpType.mult,
                )
                vr1 = wp.tile([P, H, D, 4], BF, tag="vr1")
                nc.vector.tensor_tensor(out=vr1[:], in0=p2[:, :, :, 0:4],
                                        in1=p2[:, :, :, 4:8], op=mybir.AluOpType.add)
                st["vr1", b] = vr1

            def S4den(b):
                # den[n,h] = sum_g ex[n,h,g] as 8 tiny identity matmuls (PE)
                ex = st.pop(("ex", b))
                ps_den = pd_.tile([P, H], FP, space="PSUM", tag="den")
                for g in range(H):
                    nc.tensor.matmul(out=ps_den[:], lhsT=ident[:],
                                     rhs=ex[:, :, g],
                                     start=(g == 0), stop=(g == H - 1))
                st["ps_den", b] = ps_den

            def S4r(b):
                ps_den = st.pop(("ps_den", b))
                rden = wp.tile([P, H], FP, tag="rden")
                nc.vector.reciprocal(rden[:], ps_den[:])
                st["rden", b] = rden

            def S4b(b):
                vr1 = st.pop(("vr1", b))
                rden = st.pop(("rden", b))
                vr2 = wp.tile([P, H, D, 2], BF, tag="vr2")
                nc.gpsimd.tensor_tensor(out=vr2[:], in0=vr1[:, :, :, 0:2],
                                        in1=vr1[:, :, :, 2:4], op=mybir.AluOpType.add)
                ovr = wp.tile([P, H, D], BF, tag="ovr")
                nc.gpsimd.tensor_tensor(out=ovr[:],
                                        in0=vr2[:, :, :, 0],
                                        in1=vr2[:, :, :, 1], op=mybir.AluOpType.add)
                ov = wp.tile([P, EMB], BF, tag="ov")
                nc.gpsimd.tensor_tensor(out=ov[:].rearrange("p (h d) -> p h d", h=H),
                                        in0=ovr[:],
                                        in1=rden[:].unsqueeze(2).to_broadcast([P, H, D]),
                                        op=mybir.AluOpType.mult)
                st["ov", b] = ov

            def S5(b):
                ov = st.pop(("ov", b))
                tp = pt.tile([P, 2 * P], BF, space="PSUM", tag="tp")
                nc.tensor.transpose(tp[:, 0:P], ov[:, 0:P], ident[:])
                nc.tensor.transpose(tp[:, P:2 * P], ov[:, P:2 * P], ident[:])
                ovT = wp.tile([P, 2 * P], BF, tag="ovT")
                nc.scalar.copy(ovT[:], tp[:])
                ps_oT = po.tile([P, 2 * P], FP, space="PSUM", tag="oT")
                # out^T[c',n] = sum_ch WcT[ch,c'] ovT[ch,n]  (+ bc[c'] x ones)
                nc.tensor.matmul(out=ps_oT[:, 0:P], lhsT=wca[:, 0:P], rhs=ovT[:, 0:P], start=True, stop=False)
                nc.tensor.matmul(out=ps_oT[:, 0:P], lhsT=wcb[:, 0:P], rhs=ovT[:, P:2 * P], start=False, stop=False)
                nc.tensor.matmul(out=ps_oT[:, 0:P], lhsT=btiles["bc"][:, 0:P], rhs=ones1[:], start=False, stop=True)
                nc.tensor.matmul(out=ps_oT[:, P:2 * P], lhsT=wca[:, P:EMB], rhs=ovT[:, 0:P], start=True, stop=False)
                nc.tensor.matmul(out=ps_oT[:, P:2 * P], lhsT=wcb[:, P:EMB], rhs=ovT[:, P:2 * P], start=False, stop=False)
                nc.tensor.matmul(out=ps_oT[:, P:2 * P], lhsT=btiles["bc"][:, P:EMB], rhs=ones1[:], start=False, stop=True)
                finT = wp.tile([P, 2, P], FP, tag="finT")
                nc.scalar.copy(finT[:], ps_oT[:].rearrange("p (x n) -> p x n", x=2))
                nc.sync.dma_start(outT_d[:, b, :, :], finT[:])

            DEPTH = 7
            stages = [(5, S4r), (0, S0), (1, S1), (1, S2), (2, S3), (2, S3p),
                      (3, S3f), (4, S4), (4, S4den), (5, S4b), (6, S5)]
            for i in range(NB + DEPTH - 1):
                for off, fn in stages:
                    bb = i - off
                    if 0 <= bb < NB:
                        fn(bb)

    if split_waits:
        _split_sync_waits(nc)
    return nc


# --------------------------------------------------------------- host prep
def _prep(q, k, v, edge_index, Wq, bq, Wk, bk, Wv, bv, Wc, bc):
    A = np.asarray(edge_index[0], dtype=np.int64)
    B = np.asarray(edge_index[1], dtype=np.int64)
    order = np.argsort(A, kind="stable")
    A_s = A[order]
    B_s = B[order]

    core_lo = np.searchsorted(A_s, np.arange(NCORES) * NPC, side="left")
    core_hi = np.searchsorted(A_s, (np.arange(NCORES) + 1) * NPC, side="left")

    counts = np.zeros((NCORES, NB), dtype=np.int64)
    per_core = []
    for o in range(NCORES):
        a = A_s[core_lo[o]:core_hi[o]] - o * NPC
        bi = B_s[core_lo[o]:core_hi[o]]
        blk = a // P
        counts[o] = np.bincount(blk, minlength=NB)
        per_core.append((a, bi, np.searchsorted(blk, np.arange(NB + 1))))
    tiles_per_block = np.maximum(1, (counts.max(axis=0) + P - 1) // P).astype(int)
    ET = int(tiles_per_block.sum())

    # --- per-(block,tile) destination windows, common across cores
    # slot (p, t) of block b holds the (t*128+p)-th dest-sorted edge
    lo = np.full((NB, int(tiles_per_block.max())), P, dtype=np.int64)
    hi = np.full((NB, int(tiles_per_block.max())), -1, dtype=np.int64)
    core_slot = []   # per core: (block, tile, part, dest_local, src) arrays
    for o in range(NCORES):
        a, bi, bounds = per_core[o]
        blks, tls, prts, dls, srcs = [], [], [], [], []
        for blk in range(NB):
            l, h = bounds[blk], bounds[blk + 1]
            n = h - l
            if n == 0:
                continue
            dl = a[l:h] - blk * P
            idx = np.arange(n)
            t = idx // P
            p = idx % P
            blks.append(np.full(n, blk)); tls.append(t); prts.append(p)
            dls.append(dl); srcs.append(bi[l:h])
            np.minimum.at(lo[blk], t, dl)
            np.maximum.at(hi[blk], t, dl)
        core_slot.append(tuple(np.concatenate(x) for x in
                               (blks, tls, prts, dls, srcs)))

    # tight destination windows per tile (the PSUM region is zeroed by a
    # full-width zero matmul, so windows only need to cover actual edges)
    windows = []
    for blk in range(NB):
        wb = []
        T = int(tiles_per_block[blk])
        for t in range(T):
            if hi[blk, t] < 0:
                wb.append((0, 0))
                continue
            doff = int(lo[blk, t])
            W = int(hi[blk, t]) - doff + 1
            W = min((W + 3) // 4 * 4, P - doff)
            wb.append((doff, W))
        windows.append(wb)
    SW = [sum(w for _, w in wb) for wb in windows]
    OHW = int(sum(SW))
    # column offset of tile (b,t) inside the packed one-hot stream
    oh_col = np.zeros((NB, int(tiles_per_block.max())), dtype=np.int64)
    acc = 0
    for blk in range(NB):
        for t in range(int(tiles_per_block[blk])):
            oh_col[blk, t] = acc
            acc += windows[blk][t][1]

    doffs = np.zeros((NB, int(tiles_per_block.max())), dtype=np.int64)
    for blk in range(NB):
        for t in range(int(tiles_per_block[blk])):
            doffs[blk, t] = windows[blk][t][0]

    # fused per-block stream: [ke tiles (T*EMB) | one-hot (SW[b])] per block
    KOW = [int(tiles_per_block[b]) * EMB + SW[b] for b in range(NB)]
    ko_off = np.zeros(NB + 1, dtype=np.int64)
    ko_off[1:] = np.cumsum(KOW)
    ke_base = ko_off[:NB]                       # ke part starts at block base
    oh_base = ko_off[:NB] + tiles_per_block * EMB

    k_f8 = np.asarray(k, np.float32).astype(NP_F8)
    kos = []
    for o in range(NCORES):
        blks, tls, prts, dls, srcs = core_slot[o]
        ko = np.zeros((P, int(ko_off[-1])), dtype=NP_F8)
        # scatter k rows: block-local tile t occupies [ke_base+t*EMB, ...)
        cstart = ke_base[blks] + tls * EMB
        cidx = cstart[:, None] + np.arange(EMB)[None, :]
        ko[prts[:, None], cidx] = k_f8[srcs]
        # one-hot ones: block-local window column + in-window position
        cols = (oh_base[blks] + (oh_col[blks, tls] - oh_col[blks, 0])
                + (dls - doffs[blks, tls]))
        ko[prts, cols] = 1.0
        kos.append(ko)

    cnt_nodes = np.bincount(A, minlength=N_NODES).astype(np.float32)
    invc_full = 1.0 / np.maximum(cnt_nodes, 1.0)
    invcs, cnts = [], []
    for o in range(NCORES):
        s = np.ones(NPC_PAD, dtype=np.float32)
        s[:NPC] = invc_full[o * NPC:(o + 1) * NPC]
        invcs.append(np.ascontiguousarray(s.reshape(NB, P).T))
        c = np.zeros(NPC_PAD, dtype=np.float32)
        c[:NPC] = cnt_nodes[o * NPC:(o + 1) * NPC]
        cnts.append(c.reshape(1, NPC_PAD).astype(NP_BF))

    q = np.asarray(q, dtype=np.float32)
    v = np.asarray(v, dtype=np.float32)
    qvs = []
    for o in range(NCORES):
        # qv4[p, b, j, n]: j = (q ch-lo, v ch-lo, q ch-hi, v ch-hi)
        qv = np.zeros((P, NB, 4, P), dtype=NP_BF)
        qT = np.zeros((EMB, NPC_PAD), dtype=NP_BF)
        vT = np.zeros((EMB, NPC_PAD), dtype=NP_BF)
        qT[:, :NPC] = q[o * NPC:(o + 1) * NPC].astype(NP_BF).T
        vT[:, :NPC] = v[o * NPC:(o + 1) * NPC].astype(NP_BF).T
        qv[:, :, 0, :] = qT[0:P].reshape(P, NB, P)
        qv[:, :, 1, :] = vT[0:P].reshape(P, NB, P)
        qv[:, :, 2, :] = qT[P:EMB].reshape(P, NB, P)
        qv[:, :, 3, :] = vT[P:EMB].reshape(P, NB, P)
        qvs.append(qv)

    # Wv column permutation: vh lands as [n, (d, g)]
    WvT = np.ascontiguousarray(np.asarray(Wv, np.float32).T)
    WvT_perm = WvT.reshape(EMB, H, D).transpose(0, 2, 1).reshape(EMB, EMB)
    bv_perm = np.asarray(bv, np.float32).reshape(H, D).T.reshape(-1)

    com = {
        "WqT": np.ascontiguousarray(np.asarray(Wq, np.float32).T).astype(NP_BF),
        "WkT": np.ascontiguousarray(np.asarray(Wk, np.float32).T).astype(NP_BF),
        "WvT": np.ascontiguousarray(WvT_perm).astype(NP_BF),
        "WcT": np.ascontiguousarray(np.asarray(Wc, np.float32).T).astype(NP_BF),
        "bq": np.asarray(bq, np.float32).reshape(1, EMB).astype(NP_BF),
        "bk": np.asarray(bk, np.float32).reshape(1, EMB).astype(NP_BF),
        "bv": bv_perm.reshape(1, EMB).astype(NP_BF),
        "bc": np.asarray(bc, np.float32).reshape(1, EMB).astype(NP_BF),
    }
    in_maps = []
    for o in range(NCORES):
        m = dict(com)
        m["qv"] = qvs[o]
        m["ko"] = kos[o]
        m["cnt"] = cnts[o]
        m["invc"] = invcs[o]
        in_maps.append(m)
    return tiles_per_block.tolist(), windows, in_maps


_LAST = {}


def kernel(q, k, v, edge_index, Wq, bq, Wk, bk, Wv, bv, Wc, bc, latent=None,
           _want_results=False, _trace=False):
    tiles_per_block, windows, in_maps = _prep(q, k, v, edge_index,
                                              Wq, bq, Wk, bk, Wv, bv, Wc, bc)
    key = str((tiles_per_block, windows))
    if _LAST.get("key") != key:
        _LAST["nc"] = build_nc(tiles_per_block, windows)
        _LAST["key"] = key
    nc = _LAST["nc"]

    res = run_bass_kernel_spmd(nc, in_maps, core_ids=list(range(NCORES)),
                               trace=_trace)
    out = np.empty((N_NODES, EMB), dtype=np.float32)
    for o in range(NCORES):
        oT = res.results[o]["outT"]          # [P, NB, 2, P]
        full = np.empty((EMB, NPC_PAD), dtype=np.float32)
        full[0:P] = oT[:, :, 0, :].reshape(P, NPC_PAD)
        full[P:EMB] = oT[:, :, 1, :].reshape(P, NPC_PAD)
        out[o * NPC:(o + 1) * NPC] = full[:, :NPC].T
    if _want_results:
        return out, res
    return out



# revision 14
# speedup vs baseline: 1.1985x; 1.1985x over previous
"""Trainium2 Bass kernel for GNN multi-head cross-attention message passing.

Math (see reference): per edge e: score[e,h,g] = qh[A[e],h,:] . kh[B[e],g,:]
segment-MEAN over destination A -> softmax over g -> att @ vh -> Wc projection.

Algebraic structure (as v1/v2):
  sums[n,h,g] = qh[n,h,:] . S[n,g,:],  S = (segment_sum of raw k rows) @ Wk^T
so the [E,H,H] tensor is never materialized and k is projected after
aggregation.

v4 — rebalanced against the TimelineSim cost model. Key model facts: PE
matmul costs out_free x 0.4167ns (x0.5 for fp8 DoubleRow); DVE
tensor_tensor runs 2x with all-2-byte packed operands (0.52ns/elem); Pool
(gpsimd) mult runs at 0.42 efficiency (~2ns/elem, and CANNOT touch PSUM);
ACT is 0.83ns/elem + ~185ns init per op; DMA is one shared device at
~0.356ns/partition-byte; HWDGE costs ~625ns per dma_start; PSUM is 8 banks
of 2KB with allocation rounded to banks per pool buffer.

Numerics: fp8 per-tensor noise (~3.6% rms) does NOT average away in
matmuls, so fp8 is used only where damped or residual-corrected:
 - score path (q, Wq, k, Wk, U) is all fp8: its noise enters through the
   softmax argument |mean|~0.1, so its output contribution is ~0.4%
 - V path: v and Wv ship as fp8 VALUE + fp8 RESIDUAL pairs at matched
   power-of-2 scales; vh = v8@W8 + v8@Rw8 + r8@W8 gives ~0.6% at
   DoubleRow speed; ov^T and Wc are bf16; output is fp16

Structure:
 - edges packed TWO-DEEP per destination: one one-hot column drives a
   DoubleRow matmul whose moving operand broadcasts over the k-tile pair
   (0-stride dim): half the S1 matmuls of v2 at the same DMA bytes
 - q/v/residual stream in one fp8 tensor (768B/part/block), k-rows+one-hot
   fused per block; block PAIRS share one dma_start per stream
 - qh/vh/S projections write one [P,768] PSUM region at a COMMON x1024
   scale (uT copied at x8 so S lands x1024): ONE merged ACT copy
 - score d-reduction: 32 accumulating identity matmuls; exp reads PSUM,
   scheduled LAST in ACT's program order so its wait on the same-iteration
   d-reduce does not delay the other copies
 - V-phase g-reduction runs on the PE as 16 TRANSPOSING identity matmuls
   accumulating ov^T[(h,d), n] in PSUM (g-sum and transpose are the same
   instructions); out-projection consumes ov^T directly
 - softmax: den on DVE (X-reduce), recip on DVE, att-normalize on Pool
 - DVE/Pool split of the two 2048-element product tensors via g-slices
 - PSUM banks: uT(2) + qs(2) + sc(1) + ov(2) + o(1) = 8
"""

import numpy as np
import ml_dtypes

import concourse.bass as bass
import concourse.mybir as mybir
import concourse.tile as tile
from concourse.bass_utils import run_bass_kernel_spmd
from concourse.masks import make_identity

# ---------------------------------------------------------------- constants
NCORES = 8
N_NODES = 50000
EMB = 256
H = 8
D = 32
P = 128

NPC = N_NODES // NCORES          # 6250 nodes per core
NB = (NPC + P - 1) // P          # 49 blocks of 128 nodes per core
NPC_PAD = NB * P                 # 6272

FP = mybir.dt.float32
BF = mybir.dt.bfloat16
F16 = mybir.dt.float16
F8 = mybir.dt.float8e4
DR = mybir.MatmulPerfMode.DoubleRow

NP_BF = ml_dtypes.bfloat16
NP_F8 = ml_dtypes.float8_e4m3fn

# power-of-2 fp8 scaling (exact): stream = true * SCALE
SQ = 8.0        # q, v (+ residual), k edge rows
SW_ = 128.0     # fp8 weight matrices
SPROJ = SQ * SW_   # common scale of the merged qh/vh/S PSUM region

# Pool g-shares of the two 2048-elem product tensors
PG_PROD = 2
PG_P2 = 1


# ------------------------------------------------------- sync-wait splitting
# The staged walrus accepts only ONE sync-wait command per instruction.
# Tile attaches several waits to some instructions.  Post-pass: hoist all but
# one wait of each over-limit instruction onto same-engine Drain carriers
# placed immediately before it (engine streams execute in block order, so
# "all waits hold before the instruction runs" is preserved).
_WS_COUNTER = [0]


def _split_sync_waits(nc, maxw=1):
    for f in nc.m.functions:
        for blk in f.blocks:
            insts = blk.instructions
            out = []
            changed = False
            for ins in insts:
                si = ins.sync_info
                if si is not None and len(si.on_wait) > maxw:
                    waits = list(si.on_wait)
                    k = len(waits) - maxw
                    for i in range(0, k, maxw):
                        _WS_COUNTER[0] += 1
                        d = mybir.InstDrain(
                            name=f"I-wsplit-{_WS_COUNTER[0]}", ins=[], outs=[]
                        )
                        d.engine = ins.engine
                        d.sync_info = mybir.SyncInfo(
                            on_wait=waits[i : i + maxw], on_update=[]
                        )
                        out.append(d)
                    si.on_wait = waits[k:]
                    changed = True
                out.append(ins)
            if changed:
                blk.instructions = out


# ------------------------------------------------------------- device kernel
def build_nc(pairs_per_block, windows, bias_flags, split_waits=True):
    """Build the SPMD Bass module.

    pairs_per_block[b] = 2-deep edge pair-tiles in block b (same across
    cores).  windows[b] = list of (doff, W) per pair-tile (cross-core
    union).  bias_flags = (has_bq, has_bk, has_bv, has_bc).
    """
    SW = [int(sum(w for _, w in wb)) for wb in windows]   # one-hot cols/block
    has_bq, has_bk, has_bv, has_bc = bias_flags

    nc = bass.Bass("TRN2", target_bir_lowering=False, debug=False,
                   num_devices=NCORES)

    # per-core inputs; qvr/ko/out DMA in 2-block pairs (single shared HWDGE
    # queue at ~625ns per dma_start)
    # qvr j-dim: (q8-lo, q8-hi, v8-lo, v8-hi, r8-lo, r8-hi), all *SQ
    qvr_d = nc.dram_tensor("qvr", [P, NB, 6, P], F8, kind="ExternalInput")
    KOW = [int(pairs_per_block[b]) * 2 * EMB + SW[b] for b in range(NB)]
    ko_d = nc.dram_tensor("ko", [P, sum(KOW)], F8, kind="ExternalInput")
    # fp8 weights [ch%128, ch//128, out] * SW_
    Wq8 = nc.dram_tensor("Wq8", [P, 2, EMB], F8, kind="ExternalInput")
    Wk8 = nc.dram_tensor("Wk8", [P, 2, EMB], F8, kind="ExternalInput")
    Wv8 = nc.dram_tensor("Wv8", [P, 2, EMB], F8, kind="ExternalInput")  # perm
    Rv8 = nc.dram_tensor("Rv8", [P, 2, EMB], F8, kind="ExternalInput")  # perm
    # bf16 Wc^T [(h,d)%128, (h,d)//128, c'] true scale
    Wc16 = nc.dram_tensor("Wc16", [P, 2, EMB], BF, kind="ExternalInput")
    invc_d = nc.dram_tensor("invc", [P, NB], FP, kind="ExternalInput")
    if has_bq or has_bv:
        bqv_d = nc.dram_tensor("bqv", [1, 2 * EMB], BF, kind="ExternalInput")
    if has_bk:
        bk_d = nc.dram_tensor("bk", [1, EMB], BF, kind="ExternalInput")
        cnt_d = nc.dram_tensor("cnt", [1, NPC_PAD], BF, kind="ExternalInput")
    if has_bc:
        bc_d = nc.dram_tensor("bc", [1, EMB], BF, kind="ExternalInput")

    outT_d = nc.dram_tensor("outT", [P, NB, 2, P], F16, kind="ExternalOutput")

    ko_off = [0]
    for b in range(NB):
        ko_off.append(ko_off[-1] + KOW[b])
    KOW2MAX = max(KOW[b] + (KOW[b + 1] if b + 1 < NB else 0)
                  for b in range(0, NB, 2))

    with tile.TileContext(nc) as tc:
        with (
            tc.tile_pool(name="const", bufs=1) as cp,
            tc.tile_pool(name="work", bufs=4) as wp,
            tc.tile_pool(name="qvl", bufs=8) as ql,
            tc.tile_pool(name="kep", bufs=3) as kp,
            tc.tile_pool(name="prd", bufs=3) as pr,
            tc.tile_pool(name="ps_u", bufs=2, space="PSUM") as pu,
            tc.tile_pool(name="ps_qs", bufs=1, space="PSUM") as pqs,
            tc.tile_pool(name="ps_sc", bufs=1, space="PSUM") as psc,
            tc.tile_pool(name="ps_ov", bufs=2, space="PSUM") as pov,
            tc.tile_pool(name="ps_o", bufs=1, space="PSUM") as po,
        ):
            # ---------------- constants
            ident = cp.tile([P, P], BF)
            make_identity(nc, ident[:])
            zf82 = cp.tile([P, 2, P], F8)
            nc.vector.memset(zf82[:], 0.0)
            if has_bq or has_bv or has_bk or has_bc:
                ones1 = cp.tile([1, P], BF)
                nc.vector.memset(ones1[:], 1.0)

            wt = {}
            for nm, t, dt_ in (("Wq", Wq8, F8), ("Wk", Wk8, F8),
                               ("Wv", Wv8, F8), ("Rv", Rv8, F8),
                               ("Wc", Wc16, BF)):
                s = cp.tile([P, 2, EMB], dt_, tag=f"w{nm}")
                nc.sync.dma_start(s[:], t[:])
                wt[nm] = s
            invc_sb = cp.tile([P, NB], FP)
            nc.sync.dma_start(invc_sb[:], invc_d[:])
            if has_bq or has_bv:
                bqv_sb = cp.tile([1, 2 * EMB], BF, tag="bqv")
                nc.sync.dma_start(bqv_sb[:], bqv_d[:])
            if has_bk:
                bk_sb = cp.tile([1, EMB], BF, tag="bk")
                nc.sync.dma_start(bk_sb[:], bk_d[:])
                cnt_sb = cp.tile([1, NPC_PAD], BF)
                nc.sync.dma_start(cnt_sb[:], cnt_d[:])
            if has_bc:
                bc_sb = cp.tile([1, EMB], BF, tag="bc")
                nc.sync.dma_start(bc_sb[:], bc_d[:])

            st = {}

            # ---------------- stages (software pipeline over blocks)
            def S0(b):      # SP: fetch blocks b, b+1 (b even)
                qvr = ql.tile([P, 2, 6, P], F8, tag="qvr")
                hi = min(b + 2, NB)
                nc.sync.dma_start(qvr[:, 0:hi - b, :, :], qvr_d[:, b:hi, :, :])
                ko = kp.tile([P, KOW2MAX], F8, tag="ko")
                w = ko_off[hi] - ko_off[b]
                nc.sync.dma_start(ko[:, 0:w], ko_d[:, ko_off[b]:ko_off[hi]])
                st["qvr", b] = qvr
                st["ko", b] = ko

            def S1(b):      # PE: U^T accumulation (DoubleRow, shared one-hot)
                be = b - (b % 2)
                ko = st["ko", be]
                if b % 2 == 1:
                    st.pop(("ko", be))
                base = ko_off[b] - ko_off[be]
                T = int(pairs_per_block[b])
                ke = ko[:, base:base + T * 2 * EMB].rearrange(
                    "p (t two c) -> p t two c", two=2, c=EMB)
                oh = ko[:, base + T * 2 * EMB:base + T * 2 * EMB + SW[b]]
                ps_uT = pu.tile([P, 2, P], FP, space="PSUM", tag="uT")
                for hf in range(2):
                    nc.tensor.matmul(
                        out=ps_uT[:, hf, :], lhsT=zf82[:],
                        rhs=zf82[:], start=True, stop=False,
                        perf_mode=DR, skip_group_check=True)
                wo = 0
                for t in range(T):
                    doff, W = windows[b][t]
                    last = t == T - 1
                    if W > 0:
                        ohb = oh[:, wo:wo + W].unsqueeze(1).to_broadcast(
                            [P, 2, W])
                        for hf in range(2):
                            nc.tensor.matmul(
                                out=ps_uT[:, hf, doff:doff + W],
                                lhsT=ke[:, t, :, hf * P:(hf + 1) * P],
                                rhs=ohb,
                                start=False, stop=last and hf == 1,
                                perf_mode=DR, skip_group_check=True)
                    wo += W
                st["ps_uT", b] = ps_uT

            def S1c(b):     # ACT: U^T -> SBUF fp8 (stays *SQ; prev iter)
                ps_uT = st.pop(("ps_uT", b))
                uT_sb = wp.tile([P, 2, P], F8, tag="uT_sb")
                nc.scalar.copy(uT_sb[:], ps_uT[:])
                st["uT_sb", b] = uT_sb

            def S2(b):      # PE: q/v/S projections -> one x1024 PSUM region
                qvr = st["qvr", b - (b % 2)]
                if b % 2 == 1:
                    st.pop(("qvr", b - 1))
                j = b % 2
                ps_qs = pqs.tile([P, 3 * EMB], FP, space="PSUM", tag="qs")
                nc.tensor.matmul(out=ps_qs[:, 0:EMB],
                                 lhsT=qvr[:, j, 0:2, :], rhs=wt["Wq"][:],
                                 start=True, stop=not has_bq, perf_mode=DR)
                if has_bq:
                    nc.tensor.matmul(out=ps_qs[:, 0:EMB], lhsT=ones1[:],
                                     rhs=bqv_sb[:, 0:EMB],
                                     start=False, stop=True)
                nc.tensor.matmul(out=ps_qs[:, EMB:2 * EMB],
                                 lhsT=qvr[:, j, 2:4, :], rhs=wt["Wv"][:],
                                 start=True, stop=False, perf_mode=DR)
                nc.tensor.matmul(out=ps_qs[:, EMB:2 * EMB],
                                 lhsT=qvr[:, j, 2:4, :], rhs=wt["Rv"][:],
                                 start=False, stop=False, perf_mode=DR)
                nc.tensor.matmul(out=ps_qs[:, EMB:2 * EMB],
                                 lhsT=qvr[:, j, 4:6, :], rhs=wt["Wv"][:],
                                 start=False, stop=not has_bv, perf_mode=DR)
                if has_bv:
                    nc.tensor.matmul(out=ps_qs[:, EMB:2 * EMB], lhsT=ones1[:],
                                     rhs=bqv_sb[:, EMB:2 * EMB],
                                     start=False, stop=True)
                uT_sb = st.pop(("uT_sb", b))
                nc.tensor.matmul(out=ps_qs[:, 2 * EMB:3 * EMB],
                                 lhsT=uT_sb[:], rhs=wt["Wk"][:],
                                 start=True, stop=not has_bk, perf_mode=DR)
                if has_bk:
                    nc.tensor.matmul(
                        out=ps_qs[:, 2 * EMB:3 * EMB],
                        lhsT=cnt_sb[:, b * P:(b + 1) * P],
                        rhs=bk_sb[:], start=False, stop=True)
                st["ps_qs", b] = ps_qs

            def S2c(b):     # ACT: merged qh/vh/S copy (same iter, one op)
                ps_qs = st.pop(("ps_qs", b))
                qs_sb = ql.tile([P, 3 * EMB], BF, tag="qs_sb")
                nc.scalar.mul(qs_sb[:], ps_qs[:], 1.0 / SPROJ)
                st["qs_sb", b] = qs_sb

            def S3b(b):     # DVE+Pool: prod[n,h,g,d] = qh (x) S
                qs_sb = st[("qs_sb", b)]
                qh = qs_sb[:, 0:EMB].rearrange("p (h d) -> p h d", h=H)
                sg = qs_sb[:, 2 * EMB:3 * EMB].rearrange(
                    "p (g d) -> p g d", g=H)
                prod = pr.tile([P, H, H, D], BF, tag="prod")
                gs = H - PG_PROD
                nc.vector.tensor_tensor(
                    out=prod[:, :, 0:gs, :],
                    in0=qh.unsqueeze(2).to_broadcast([P, H, gs, D]),
                    in1=sg[:, 0:gs, :].unsqueeze(1).to_broadcast(
                        [P, H, gs, D]),
                    op=mybir.AluOpType.mult)
                nc.gpsimd.tensor_tensor(
                    out=prod[:, :, gs:H, :],
                    in0=qh.unsqueeze(2).to_broadcast([P, H, PG_PROD, D]),
                    in1=sg[:, gs:H, :].unsqueeze(1).to_broadcast(
                        [P, H, PG_PROD, D]),
                    op=mybir.AluOpType.mult)
                st["prod", b] = prod

            def S3p(b):     # PE: d-reduction (32 matmuls)
                prod = st.pop(("prod", b))
                ps_sc = psc.tile([P, H, H], FP, space="PSUM", tag="sc")
                for dd in range(D):
                    nc.tensor.matmul(out=ps_sc[:], lhsT=ident[:],
                                     rhs=prod[:, :, :, dd],
                                     start=(dd == 0), stop=(dd == D - 1))
                st["ps_sc", b] = ps_sc

            def S3f(b):     # ACT: ex = exp(sums * invc), last in ACT program
                ps_sc = st.pop(("ps_sc", b))
                ex = wp.tile([P, H, H], BF, tag="ex")
                nc.scalar.activation(out=ex[:], in_=ps_sc[:],
                                     func=mybir.ActivationFunctionType.Exp,
                                     scale=invc_sb[:, b:b + 1])
                st["ex", b] = ex

            def S4r(b):     # DVE: den + rden; Pool: att = ex * rden
                ex = st.pop(("ex", b))
                den = wp.tile([P, H], FP, tag="den")
                nc.vector.tensor_reduce(den[:], ex[:],
                                        axis=mybir.AxisListType.X,
                                        op=mybir.AluOpType.add)
                rden = wp.tile([P, H], BF, tag="rden")
                with nc.allow_low_precision(
                        reason="bf16 softmax normalization, 2^-8 rel err"):
                    nc.vector.reciprocal(rden[:], den[:])
                att = wp.tile([P, H, H], BF, tag="att")
                nc.gpsimd.tensor_tensor(
                    out=att[:], in0=ex[:],
                    in1=rden[:].unsqueeze(2).to_broadcast([P, H, H]),
                    op=mybir.AluOpType.mult)
                st["att", b] = att

            def S4b(b):     # DVE+Pool: p2[n,h,d,g] = att (x) vh
                qs_sb = st.pop(("qs_sb", b))
                att = st.pop(("att", b))
                vh = qs_sb[:, EMB:2 * EMB].rearrange("p (d g) -> p d g", d=D)
                p2 = pr.tile([P, H, D, H], BF, tag="p2")
                gs = H - PG_P2
                nc.vector.tensor_tensor(
                    out=p2[:, :, :, 0:gs],
                    in0=att[:, :, 0:gs].unsqueeze(2).to_broadcast(
                        [P, H, D, gs]),
                    in1=vh[:, :, 0:gs].unsqueeze(1).to_broadcast(
                        [P, H, D, gs]),
                    op=mybir.AluOpType.mult)
                nc.gpsimd.tensor_tensor(
                    out=p2[:, :, :, gs:H],
                    in0=att[:, :, gs:H].unsqueeze(2).to_broadcast(
                        [P, H, D, PG_P2]),
                    in1=vh[:, :, gs:H].unsqueeze(1).to_broadcast(
                        [P, H, D, PG_P2]),
                    op=mybir.AluOpType.mult)
                st["p2", b] = p2

            def S5a(b):     # PE: transposing g-sum -> ov^T[(h,d), n] in PSUM
                p2 = st.pop(("p2", b))
                ps_ov = pov.tile([P, 2, P], FP, space="PSUM", tag="ov")
                for hf in range(2):
                    for g in range(H):
                        lhsT = p2[:, 4 * hf:4 * hf + 4, :, g].rearrange(
                            "p h d -> p (h d)")
                        nc.tensor.matmul(out=ps_ov[:, hf, :], lhsT=lhsT,
                                         rhs=ident[:],
                                         start=(g == 0), stop=(g == H - 1),
                                         skip_group_check=True)
                st["ps_ov", b] = ps_ov

            def S5b(b):     # ACT: ov^T -> SBUF bf16 (prev iter)
                ps_ov = st.pop(("ps_ov", b))
                ovT = wp.tile([P, 2, P], BF, tag="ovT")
                nc.scalar.copy(ovT[:], ps_ov[:])
                st["ovT", b] = ovT

            def S5c(b):     # PE: out^T = Wc^T @ ov^T (bf16)
                ovT = st.pop(("ovT", b))
                ps_o = po.tile([P, 2, P], FP, space="PSUM", tag="o")
                for cf in range(2):
                    for hh in range(2):
                        nc.tensor.matmul(
                            out=ps_o[:, cf, :],
                            lhsT=wt["Wc"][:, hh, cf * P:(cf + 1) * P],
                            rhs=ovT[:, hh, :],
                            start=(hh == 0),
                            stop=(hh == 1) and not has_bc,
                            skip_group_check=True)
                    if has_bc:
                        nc.tensor.matmul(
                            out=ps_o[:, cf, :],
                            lhsT=bc_sb[:, cf * P:(cf + 1) * P], rhs=ones1[:],
                            start=False, stop=True, skip_group_check=True)
                st["ps_o", b] = ps_o

            def S5d(b):     # ACT: fp16 out; SP: DMA pair on odd b
                ps_o = st.pop(("ps_o", b))
                if b % 2 == 0:
                    fo = ql.tile([P, 2, 2, P], F16, tag="fo")
                    st["fo", b] = fo
                else:
                    fo = st[("fo", b - 1)]
                nc.scalar.copy(fo[:, b % 2, :, :], ps_o[:])
                if b % 2 == 1 or b == NB - 1:
                    be = b - (b % 2)
                    fo = st.pop(("fo", be))
                    hi = min(be + 2, NB)
                    nc.sync.dma_start(outT_d[:, be:hi, :, :],
                                      fo[:, 0:hi - be, :, :])

            # list order = per-engine program order; exp (S3f) is LAST so
            # its same-iteration wait on the d-reduce doesn't delay the
            # other ACT copies
            stages = [(0, S0), (2, S1), (3, S1c), (3, S2), (3, S2c),
                      (4, S3b), (5, S3p), (6, S4r), (7, S4b), (8, S5a),
                      (9, S5b), (10, S5c), (11, S5d), (5, S3f)]
            DEPTH = 12
            for i in range(NB + DEPTH - 1):
                for off, fn in stages:
                    bb = i - off
                    if 0 <= bb < NB:
                        if fn is S0 and bb % 2 == 1:
                            continue
                        fn(bb)

    if split_waits:
        _split_sync_waits(nc)
    return nc


# --------------------------------------------------------------- host prep
def _prep(q, k, v, edge_index, Wq, bq, Wk, bk, Wv, bv, Wc, bc):
    A = np.asarray(edge_index[0], dtype=np.int64)
    B = np.asarray(edge_index[1], dtype=np.int64)
    order = np.argsort(A, kind="stable")
    A_s = A[order]
    B_s = B[order]

    core_lo = np.searchsorted(A_s, np.arange(NCORES) * NPC, side="left")
    core_hi = np.searchsorted(A_s, (np.arange(NCORES) + 1) * NPC, side="left")

    # --- per-core 2-deep slot assignment (vectorized)
    per_core = []
    npair = np.zeros((NCORES, NB), dtype=np.int64)
    for o in range(NCORES):
        a = A_s[core_lo[o]:core_hi[o]] - o * NPC      # local dest, ascending
        bi = B_s[core_lo[o]:core_hi[o]]
        n = len(a)
        first = np.searchsorted(a, a, side="left")
        rank = np.arange(n) - first
        depth = rank % 2
        cnt_d = np.bincount(a, minlength=NPC_PAD)
        s_d = (cnt_d + 1) // 2
        s_cum = np.cumsum(s_d) - s_d                   # global slot prefix
        blk_start = s_cum[np.arange(NB) * P]           # first slot of block
        slot_in_blk = (s_cum[a] - blk_start[a // P]) + rank // 2
        t = slot_in_blk // P
        p = slot_in_blk % P
        blk = a // P
        np.maximum.at(npair[o], blk, t + 1)
        per_core.append((a, bi, blk, t, p, depth))
    pairs_per_block = np.maximum(1, npair.max(axis=0)).astype(int)

    # --- per-(block,tile) destination windows, union across cores
    TMAX = int(pairs_per_block.max())
    lo = np.full((NB, TMAX), P, dtype=np.int64)
    hi = np.full((NB, TMAX), -1, dtype=np.int64)
    for o in range(NCORES):
        a, bi, blk, t, p, depth = per_core[o]
        dl = a - blk * P
        np.minimum.at(lo, (blk, t), dl)
        np.maximum.at(hi, (blk, t), dl)
    windows = []
    for bidx in range(NB):
        wb = []
        for t in range(int(pairs_per_block[bidx])):
            if hi[bidx, t] < 0:
                wb.append((0, 0))
                continue
            doff = int(lo[bidx, t])
            W = int(hi[bidx, t]) - doff + 1
            W = min((W + 3) // 4 * 4, P - doff)
            wb.append((doff, W))
        windows.append(wb)
    SW = [sum(w for _, w in wb) for wb in windows]

    KOW = [int(pairs_per_block[b]) * 2 * EMB + SW[b] for b in range(NB)]
    ko_off = np.zeros(NB + 1, dtype=np.int64)
    ko_off[1:] = np.cumsum(KOW)
    ke_base = ko_off[:NB]
    oh_base = ko_off[:NB] + pairs_per_block * 2 * EMB
    oh_col = np.zeros((NB, TMAX), dtype=np.int64)
    doffs = np.zeros((NB, TMAX), dtype=np.int64)
    for bidx in range(NB):
        acc = 0
        for t in range(int(pairs_per_block[bidx])):
            oh_col[bidx, t] = acc
            acc += windows[bidx][t][1]
            doffs[bidx, t] = windows[bidx][t][0]

    k8 = (np.asarray(k, np.float32) * SQ).astype(NP_F8)
    kos = []
    for o in range(NCORES):
        a, bi, blk, t, p, depth = per_core[o]
        ko = np.zeros((P, int(ko_off[-1])), dtype=NP_F8)
        cstart = ke_base[blk] + t * 2 * EMB + depth * EMB
        cidx = cstart[:, None] + np.arange(EMB)[None, :]
        ko[p[:, None], cidx] = k8[bi]
        m0 = depth == 0
        cols = (oh_base[blk[m0]] + oh_col[blk[m0], t[m0]]
                + (a[m0] - blk[m0] * P - doffs[blk[m0], t[m0]]))
        ko[p[m0], cols] = 1.0
        kos.append(ko)

    cnt_nodes = np.bincount(A, minlength=N_NODES).astype(np.float32)
    invc_full = 1.0 / np.maximum(cnt_nodes, 1.0)
    invcs, cnts = [], []
    for o in range(NCORES):
        s = np.ones(NPC_PAD, dtype=np.float32)
        s[:NPC] = invc_full[o * NPC:(o + 1) * NPC]
        invcs.append(np.ascontiguousarray(s.reshape(NB, P).T))
        c = np.zeros(NPC_PAD, dtype=np.float32)
        c[:NPC] = cnt_nodes[o * NPC:(o + 1) * NPC]
        cnts.append((c * SPROJ).reshape(1, NPC_PAD).astype(NP_BF))

    # q fp8; v as fp8 value + fp8 residual (both *SQ, exact power-of-2)
    q8 = (np.asarray(q, np.float32) * SQ).astype(NP_F8)
    vs = np.asarray(v, np.float32) * SQ
    v8 = vs.astype(NP_F8)
    r8 = (vs - v8.astype(np.float32)).astype(NP_F8)
    qvrs = []
    for o in range(NCORES):
        qvr = np.zeros((P, NB, 6, P), dtype=NP_F8)
        for j, src in ((0, q8), (2, v8), (4, r8)):
            sT = np.zeros((EMB, NPC_PAD), dtype=NP_F8)
            sT[:, :NPC] = src[o * NPC:(o + 1) * NPC].T
            qvr[:, :, j, :] = sT[0:P].reshape(P, NB, P)
            qvr[:, :, j + 1, :] = sT[P:EMB].reshape(P, NB, P)
        qvrs.append(qvr)

    # Wv column permutation: vh lands as [n, (d, g)]
    WvT = np.ascontiguousarray(np.asarray(Wv, np.float32).T)
    WvT_perm = WvT.reshape(EMB, H, D).transpose(0, 2, 1).reshape(EMB, EMB)
    bv_perm = np.asarray(bv, np.float32).reshape(H, D).T.reshape(-1)

    def pack2(WT):
        # [ch, out] -> [ch%128, ch//128, out]
        a = np.ascontiguousarray(np.asarray(WT, np.float32)).reshape(
            2, P, EMB)
        return np.ascontiguousarray(a.transpose(1, 0, 2))

    Wv_s = WvT_perm * SW_
    Wv8 = Wv_s.astype(NP_F8)
    Rv8 = (Wv_s - Wv8.astype(np.float32)).astype(NP_F8)

    bias_flags = (bool(np.any(np.asarray(bq))), bool(np.any(np.asarray(bk))),
                  bool(np.any(np.asarray(bv))), bool(np.any(np.asarray(bc))))
    has_bq, has_bk, has_bv, has_bc = bias_flags

    com = {
        "Wq8": pack2(np.asarray(Wq, np.float32).T * SW_).astype(NP_F8),
        "Wk8": pack2(np.asarray(Wk, np.float32).T * SW_).astype(NP_F8),
        "Wv8": pack2(Wv8.astype(np.float32)).astype(NP_F8),
        "Rv8": pack2(Rv8.astype(np.float32)).astype(NP_F8),
        "Wc16": pack2(np.asarray(Wc, np.float32).T).astype(NP_BF),
    }
    if has_bq or has_bv:
        bqv = np.concatenate([np.asarray(bq, np.float32),
                              bv_perm]) * SPROJ
        com["bqv"] = bqv.reshape(1, 2 * EMB).astype(NP_BF)
    if has_bk:
        com["bk"] = np.asarray(bk, np.float32).reshape(1, EMB).astype(NP_BF)
    if has_bc:
        com["bc"] = np.asarray(bc, np.float32).reshape(1, EMB).astype(NP_BF)

    in_maps = []
    for o in range(NCORES):
        m = dict(com)
        m["qvr"] = qvrs[o]
        m["ko"] = kos[o]
        m["invc"] = invcs[o]
        if has_bk:
            m["cnt"] = cnts[o]
        in_maps.append(m)
    return pairs_per_block.tolist(), windows, bias_flags, in_maps


_LAST = {}


def kernel(q, k, v, edge_index, Wq, bq, Wk, bk, Wv, bv, Wc, bc, latent=None,
           _want_results=False, _trace=False):
    pairs_per_block, windows, bias_flags, in_maps = _prep(
        q, k, v, edge_index, Wq, bq, Wk, bk, Wv, bv, Wc, bc)
    key = str((pairs_per_block, windows, bias_flags))
    if _LAST.get("key") != key:
        _LAST["nc"] = build_nc(pairs_per_block, windows, bias_flags)
        _LAST["key"] = key
    nc = _LAST["nc"]

    res = run_bass_kernel_spmd(nc, in_maps, core_ids=list(range(NCORES)),
                               trace=_trace)
    out = np.empty((N_NODES, EMB), dtype=np.float32)
    for o in range(NCORES):
        oT = res.results[o]["outT"].astype(np.float32)   # [P, NB, 2, P]
        full = np.empty((EMB, NPC_PAD), dtype=np.float32)
        full[0:P] = oT[:, :, 0, :].reshape(P, NPC_PAD)
        full[P:EMB] = oT[:, :, 1, :].reshape(P, NPC_PAD)
        out[o * NPC:(o + 1) * NPC] = full[:, :NPC].T
    if _want_results:
        return out, res
    return out


# revision 33
# speedup vs baseline: 1.2282x; 1.0248x over previous
"""Trainium2 Bass kernel for GNN multi-head cross-attention message passing.

Math (see reference): per edge e: score[e,h,g] = qh[A[e],h,:] . kh[B[e],g,:]
segment-MEAN over destination A -> softmax over g -> att @ vh -> Wc projection.

Algebraic structure (as v1/v2):
  sums[n,h,g] = qh[n,h,:] . S[n,g,:],  S = (segment_sum of raw k rows) @ Wk^T
so the [E,H,H] tensor is never materialized and k is projected after
aggregation.

v4 — rebalanced against the TimelineSim cost model. Key model facts: PE
matmul costs out_free x 0.4167ns (x0.5 for fp8 DoubleRow); DVE
tensor_tensor runs 2x with all-2-byte packed operands (0.52ns/elem); Pool
(gpsimd) mult runs at 0.42 efficiency (~2ns/elem, and CANNOT touch PSUM);
ACT is 0.83ns/elem + ~185ns init per op; DMA is one shared device at
~0.356ns/partition-byte; HWDGE costs ~625ns per dma_start; PSUM is 8 banks
of 2KB with allocation rounded to banks per pool buffer.

Numerics: fp8 per-tensor noise (~3.6% rms) does NOT average away in
matmuls, so fp8 is used only where damped or residual-corrected:
 - score path (q, Wq, k, Wk, U) is all fp8: its noise enters through the
   softmax argument |mean|~0.1, so its output contribution is ~0.4%
 - V path: v and Wv ship as fp8 VALUE + fp8 RESIDUAL pairs at matched
   power-of-2 scales; vh = v8@W8 + v8@Rw8 + r8@W8 gives ~0.6% at
   DoubleRow speed; ov^T and Wc are bf16; output is fp16

Structure:
 - edges packed TWO-DEEP per destination: one one-hot column drives a
   DoubleRow matmul whose moving operand broadcasts over the k-tile pair
   (0-stride dim): half the S1 matmuls of v2 at the same DMA bytes
 - q/v/residual stream in one fp8 tensor (768B/part/block), k-rows+one-hot
   fused per block; block PAIRS share one dma_start per stream
 - qh/vh/S projections write one [P,768] PSUM region at a COMMON x1024
   scale (uT copied at x8 so S lands x1024): ONE merged ACT copy
 - score d-reduction: 32 accumulating identity matmuls; exp reads PSUM,
   scheduled LAST in ACT's program order so its wait on the same-iteration
   d-reduce does not delay the other copies
 - V-phase g-reduction runs on the PE as 16 TRANSPOSING identity matmuls
   accumulating ov^T[(h,d), n] in PSUM (g-sum and transpose are the same
   instructions); out-projection consumes ov^T directly
 - softmax: den on DVE (X-reduce), recip on DVE, att-normalize on Pool
 - DVE/Pool split of the two 2048-element product tensors via g-slices
 - PSUM banks: uT(2) + qs(2) + sc(1) + ov(2) + o(1) = 8
"""

import numpy as np
import ml_dtypes

import concourse.bass as bass
import concourse.mybir as mybir
import concourse.tile as tile
from concourse.bass_utils import run_bass_kernel_spmd
from concourse.masks import make_identity

# ---------------------------------------------------------------- constants
NCORES = 8
N_NODES = 50000
EMB = 256
H = 8
D = 32
P = 128

NPC = N_NODES // NCORES          # 6250 nodes per core
NB = (NPC + P - 1) // P          # 49 blocks of 128 nodes per core
NPC_PAD = NB * P                 # 6272

FP = mybir.dt.float32
BF = mybir.dt.bfloat16
F16 = mybir.dt.float16
F8 = mybir.dt.float8e4
DR = mybir.MatmulPerfMode.DoubleRow

NP_BF = ml_dtypes.bfloat16
NP_F8 = ml_dtypes.float8_e4m3fn

# power-of-2 fp8 scaling (exact): stream = true * SCALE
SQ = 8.0        # q, v (+ residual), k edge rows
SW_ = 128.0     # fp8 weight matrices
SPROJ = SQ * SW_   # common scale of the merged qh/vh/S PSUM region

# Pool g-shares of the two 2048-elem product tensors
PG_PROD = 2
PG_P2 = 1


# ------------------------------------------------------- sync-wait splitting
# The staged walrus accepts only ONE sync-wait command per instruction.
# Tile attaches several waits to some instructions.  Post-pass: hoist all but
# one wait of each over-limit instruction onto same-engine Drain carriers
# placed immediately before it (engine streams execute in block order, so
# "all waits hold before the instruction runs" is preserved).
_WS_COUNTER = [0]


def _split_sync_waits(nc, maxw=1):
    for f in nc.m.functions:
        for blk in f.blocks:
            insts = blk.instructions
            out = []
            changed = False
            for ins in insts:
                si = ins.sync_info
                if si is not None and len(si.on_wait) > maxw:
                    waits = list(si.on_wait)
                    k = len(waits) - maxw
                    for i in range(0, k, maxw):
                        _WS_COUNTER[0] += 1
                        d = mybir.InstDrain(
                            name=f"I-wsplit-{_WS_COUNTER[0]}", ins=[], outs=[]
                        )
                        d.engine = ins.engine
                        d.sync_info = mybir.SyncInfo(
                            on_wait=waits[i : i + maxw], on_update=[]
                        )
                        out.append(d)
                    si.on_wait = waits[k:]
                    changed = True
                out.append(ins)
            if changed:
                blk.instructions = out


# ------------------------------------------------------------- device kernel
def build_nc(pairs_per_block, windows, bias_flags, split_waits=True):
    """Build the SPMD Bass module.

    pairs_per_block[b] = 2-deep edge pair-tiles in block b (same across
    cores).  windows[b] = list of (doff, W) per pair-tile (cross-core
    union).  bias_flags = (has_bq, has_bk, has_bv, has_bc).
    """
    SW = [int(sum(w for _, w in wb)) for wb in windows]   # one-hot cols/block
    has_bq, has_bk, has_bv, has_bc = bias_flags

    nc = bass.Bass("TRN2", target_bir_lowering=False, debug=False,
                   num_devices=NCORES)

    # per-core inputs; qvr/ko/out DMA in 2-block pairs (single shared HWDGE
    # queue at ~625ns per dma_start)
    # qvr j-dim: (q8-lo, q8-hi, v8-lo, v8-hi, r8-lo, r8-hi), all *SQ
    qvr_d = nc.dram_tensor("qvr", [P, NB, 6, P], F8, kind="ExternalInput")
    KOW = [int(pairs_per_block[b]) * 2 * EMB + SW[b] for b in range(NB)]
    ko_d = nc.dram_tensor("ko", [P, sum(KOW)], F8, kind="ExternalInput")
    # fp8 weights [ch%128, ch//128, out] * SW_
    Wq8 = nc.dram_tensor("Wq8", [P, 2, EMB], F8, kind="ExternalInput")
    Wk8 = nc.dram_tensor("Wk8", [P, 2, EMB], F8, kind="ExternalInput")
    Wv8 = nc.dram_tensor("Wv8", [P, 2, EMB], F8, kind="ExternalInput")  # perm
    Rv8 = nc.dram_tensor("Rv8", [P, 2, EMB], F8, kind="ExternalInput")  # perm
    # bf16 Wc^T [(h,d)%128, (h,d)//128, c'] true scale
    Wc16 = nc.dram_tensor("Wc16", [P, 2, EMB], BF, kind="ExternalInput")
    invc_d = nc.dram_tensor("invc", [P, NB], FP, kind="ExternalInput")
    if has_bq or has_bv:
        bqv_d = nc.dram_tensor("bqv", [1, 2 * EMB], BF, kind="ExternalInput")
    if has_bk:
        bk_d = nc.dram_tensor("bk", [1, EMB], BF, kind="ExternalInput")
        cnt_d = nc.dram_tensor("cnt", [1, NPC_PAD], BF, kind="ExternalInput")
    if has_bc:
        bc_d = nc.dram_tensor("bc", [1, EMB], BF, kind="ExternalInput")

    outT_d = nc.dram_tensor("outT", [P, NB, 2, P], F16, kind="ExternalOutput")

    ko_off = [0]
    for b in range(NB):
        ko_off.append(ko_off[-1] + KOW[b])
    KOW2MAX = max(KOW[b] + (KOW[b + 1] if b + 1 < NB else 0)
                  for b in range(0, NB, 2))

    with tile.TileContext(nc) as tc:
        with (
            tc.tile_pool(name="const", bufs=1) as cp,
            tc.tile_pool(name="work", bufs=6) as wp,
            tc.tile_pool(name="qvl", bufs=9) as ql,
            tc.tile_pool(name="kep", bufs=4) as kp,
            tc.tile_pool(name="prd", bufs=4) as pr,
            tc.tile_pool(name="ps_u", bufs=1, space="PSUM") as pu,
            tc.tile_pool(name="ps_qs", bufs=1, space="PSUM") as pqs,
            tc.tile_pool(name="ps_sc", bufs=1, space="PSUM") as psc,
            tc.tile_pool(name="ps_ov", bufs=2, space="PSUM") as pov,
            tc.tile_pool(name="ps_o", bufs=1, space="PSUM") as po,
        ):
            # ---------------- constants
            ident = cp.tile([P, P], BF)
            make_identity(nc, ident[:])
            ident82 = cp.tile([P, 2, P], F8)     # identity pair for DR d-red
            nc.scalar.copy(ident82[:, 0, :], ident[:])
            nc.scalar.copy(ident82[:, 1, :], ident[:])
            zf82 = cp.tile([P, 2, P], F8)
            nc.vector.memset(zf82[:], 0.0)
            if has_bq or has_bv or has_bk or has_bc:
                ones1 = cp.tile([1, P], BF)
                nc.vector.memset(ones1[:], 1.0)

            wt = {}
            for nm, t, dt_ in (("Wq", Wq8, F8), ("Wk", Wk8, F8),
                               ("Wv", Wv8, F8), ("Rv", Rv8, F8),
                               ("Wc", Wc16, BF)):
                s = cp.tile([P, 2, EMB], dt_, tag=f"w{nm}")
                nc.sync.dma_start(s[:], t[:])
                wt[nm] = s
            invc_sb = cp.tile([P, NB], FP)
            nc.sync.dma_start(invc_sb[:], invc_d[:])
            if has_bq or has_bv:
                bqv_sb = cp.tile([1, 2 * EMB], BF, tag="bqv")
                nc.sync.dma_start(bqv_sb[:], bqv_d[:])
            if has_bk:
                bk_sb = cp.tile([1, EMB], BF, tag="bk")
                nc.sync.dma_start(bk_sb[:], bk_d[:])
                cnt_sb = cp.tile([1, NPC_PAD], BF)
                nc.sync.dma_start(cnt_sb[:], cnt_d[:])
            if has_bc:
                bc_sb = cp.tile([1, EMB], BF, tag="bc")
                nc.sync.dma_start(bc_sb[:], bc_d[:])

            st = {}

            # ---------------- stages (software pipeline over block PAIRS)
            # Post-S1 stages process a pair of blocks per op: ACT/DVE/Pool
            # per-op init costs are paid once per pair, and PSUM pair-tiles
            # use first-writer-zeroes (start=True only on each bank's first
            # matmul group; later groups accumulate onto the pending-zero
            # bytes, the same HW-proven idiom as the S1 window resets).
            def S0(m):      # SP: fetch pair m (blocks 2m, 2m+1)
                b = 2 * m
                qvr = ql.tile([P, 2, 6, P], F8, tag="qvr")
                hi = min(b + 2, NB)
                nc.sync.dma_start(qvr[:, 0:hi - b, :, :], qvr_d[:, b:hi, :, :])
                ko = kp.tile([P, KOW2MAX], F8, tag="ko")
                w = ko_off[hi] - ko_off[b]
                nc.sync.dma_start(ko[:, 0:w], ko_d[:, ko_off[b]:ko_off[hi]])
                st["qvr", m] = qvr
                st["ko", m] = ko

            def blocks_of(m):
                b0 = 2 * m
                return [b0] if b0 + 1 >= NB else [b0, b0 + 1]

            def S1(m):      # PE: U^T pair accumulation (DR, shared one-hot)
                ko = st.pop(("ko", m))
                blks = blocks_of(m)
                ps_uT = pu.tile([P, 2, 2, P], FP, space="PSUM", tag="uT")
                # full-width zero reset (write-mode), then accumulate
                mms = []
                for j in range(2):
                    for hf in range(2):
                        mms.append(dict(out=ps_uT[:, j, hf, :], lhsT=zf82[:],
                                        rhs=zf82[:], start=True, stop=False,
                                        perf_mode=DR, skip_group_check=True))
                for b in blks:
                    base = ko_off[b] - ko_off[2 * m]
                    T = int(pairs_per_block[b])
                    ke = ko[:, base:base + T * 2 * EMB].rearrange(
                        "p (t two c) -> p t two c", two=2, c=EMB)
                    oh = ko[:, base + T * 2 * EMB:base + T * 2 * EMB + SW[b]]
                    wo = 0
                    for t in range(T):
                        doff, W = windows[b][t]
                        if W > 0:
                            ohb = oh[:, wo:wo + W].unsqueeze(1).to_broadcast(
                                [P, 2, W])
                            for hf in range(2):
                                mms.append(dict(
                                    out=ps_uT[:, b % 2, hf, doff:doff + W],
                                    lhsT=ke[:, t, :, hf * P:(hf + 1) * P],
                                    rhs=ohb, start=False, stop=False,
                                    perf_mode=DR, skip_group_check=True))
                        wo += W
                    mms[-1]["stop"] = b == blks[-1]
                for kw in mms:
                    nc.tensor.matmul(**kw)
                st["ps_uT", m] = ps_uT

            def S1c(m):     # ACT op1: U^T pair -> SBUF fp8 (stays *SQ)
                ps_uT = st.pop(("ps_uT", m))
                uT_sb = wp.tile([P, 2, 2, P], F8, tag="uT_sb")
                nc.scalar.copy(uT_sb[:], ps_uT[:])
                st["uT_sb", m] = uT_sb

            def S2(m):      # PE: q/v/S projections, pair -> one x1024 region
                qvr = st.pop(("qvr", m))
                uT_sb = st.pop(("uT_sb", m))
                ps_qs = pqs.tile([P, 2, 3 * EMB], FP, space="PSUM", tag="qs")
                for j, b in enumerate(blocks_of(m)):
                    reg = ps_qs[:, j, :]
                    nc.tensor.matmul(out=reg[:, 0:EMB],
                                     lhsT=qvr[:, j, 0:2, :], rhs=wt["Wq"][:],
                                     start=True, stop=not has_bq,
                                     perf_mode=DR, skip_group_check=True)
                    if has_bq:
                        nc.tensor.matmul(out=reg[:, 0:EMB], lhsT=ones1[:],
                                         rhs=bqv_sb[:, 0:EMB],
                                         start=False, stop=True,
                                         skip_group_check=True)
                    nc.tensor.matmul(out=reg[:, EMB:2 * EMB],
                                     lhsT=qvr[:, j, 2:4, :], rhs=wt["Wv"][:],
                                     start=True, stop=False,
                                     perf_mode=DR, skip_group_check=True)
                    nc.tensor.matmul(out=reg[:, EMB:2 * EMB],
                                     lhsT=qvr[:, j, 2:4, :], rhs=wt["Rv"][:],
                                     start=False, stop=False,
                                     perf_mode=DR, skip_group_check=True)
                    nc.tensor.matmul(out=reg[:, EMB:2 * EMB],
                                     lhsT=qvr[:, j, 4:6, :], rhs=wt["Wv"][:],
                                     start=False, stop=not has_bv,
                                     perf_mode=DR, skip_group_check=True)
                    if has_bv:
                        nc.tensor.matmul(out=reg[:, EMB:2 * EMB],
                                         lhsT=ones1[:],
                                         rhs=bqv_sb[:, EMB:2 * EMB],
                                         start=False, stop=True,
                                         skip_group_check=True)
                    nc.tensor.matmul(out=reg[:, 2 * EMB:3 * EMB],
                                     lhsT=uT_sb[:, j, :, :], rhs=wt["Wk"][:],
                                     start=True, stop=not has_bk,
                                     perf_mode=DR, skip_group_check=True)
                    if has_bk:
                        nc.tensor.matmul(
                            out=reg[:, 2 * EMB:3 * EMB],
                            lhsT=cnt_sb[:, b * P:(b + 1) * P],
                            rhs=bk_sb[:], start=False, stop=True,
                            skip_group_check=True)
                st["ps_qs", m] = ps_qs

            def S2c(m):     # ACT op2: merged qh/vh/S pair copy
                ps_qs = st.pop(("ps_qs", m))
                qs_sb = ql.tile([P, 2, 3 * EMB], BF, tag="qs_sb")
                nc.scalar.mul(qs_sb[:], ps_qs[:], 1.0 / SPROJ)
                st["qs_sb", m] = qs_sb

            def S3b(m):     # DVE: prod bf16; Pool: fp8 g-tail (per half
                            # -- ISA free-dim patterns are 3D max)
                qs_sb = st[("qs_sb", m)]
                gs = H - PG_PROD
                prod = pr.tile([P, 2, H, gs, D], BF, tag="prod")
                prod8 = pr.tile([P, 2, H, PG_PROD, D], F8, tag="prod8")
                for j in range(2):
                    qh = qs_sb[:, j, 0:EMB].rearrange(
                        "p (h d) -> p h d", h=H)
                    sg = qs_sb[:, j, 2 * EMB:3 * EMB].rearrange(
                        "p (g d) -> p g d", g=H)
                    nc.vector.tensor_tensor(
                        out=prod[:, j],
                        in0=qh.unsqueeze(2).to_broadcast([P, H, gs, D]),
                        in1=sg[:, 0:gs, :].unsqueeze(1).to_broadcast(
                            [P, H, gs, D]),
                        op=mybir.AluOpType.mult)
                    nc.gpsimd.tensor_tensor(
                        out=prod8[:, j],
                        in0=qh.unsqueeze(2).to_broadcast(
                            [P, H, PG_PROD, D]),
                        in1=sg[:, gs:H, :].unsqueeze(1).to_broadcast(
                            [P, H, PG_PROD, D]),
                        op=mybir.AluOpType.mult)
                st["prod", m] = prod
                st["prod8", m] = prod8

            def S3p(m):     # PE: pair d-red (32 bf16 + 2x16 fp8-DR)
                prod = st.pop(("prod", m))
                prod8 = st.pop(("prod8", m))
                gs = H - PG_PROD
                ps_sc = psc.tile([P, 2, H, H], FP, space="PSUM", tag="sc")
                for dd in range(D):
                    nc.tensor.matmul(out=ps_sc[:, :, :, 0:gs], lhsT=ident[:],
                                     rhs=prod[:, :, :, :, dd],
                                     start=(dd == 0), stop=False,
                                     skip_group_check=True)
                for j in range(2):
                    for jj in range(D // 2):
                        nc.tensor.matmul(
                            out=ps_sc[:, j, :, gs:H],
                            lhsT=ident82[:],
                            rhs=prod8[:, j, :, :, 2 * jj:2 * jj + 2
                                      ].rearrange("p h g d -> p d (h g)"),
                            start=(jj == 0),
                            stop=(jj == D // 2 - 1) and j == 1,
                            perf_mode=DR, skip_group_check=True)
                st["ps_sc", m] = ps_sc

            def S3f(m):     # ACT (last): per-block exp from PSUM
                ps_sc = st.pop(("ps_sc", m))
                ex = wp.tile([P, 2, H, H], BF, tag="ex")
                for j, b in enumerate(blocks_of(m)):
                    nc.scalar.activation(
                        out=ex[:, j, :, :], in_=ps_sc[:, j, :, :],
                        func=mybir.ActivationFunctionType.Exp,
                        scale=invc_sb[:, b:b + 1])
                st["ex", m] = ex

            def S4r(m):     # DVE: pair den + rden; Pool: att = ex * rden
                ex = st.pop(("ex", m))
                den = wp.tile([P, 2, H], FP, tag="den")
                nc.vector.tensor_reduce(den[:], ex[:],
                                        axis=mybir.AxisListType.X,
                                        op=mybir.AluOpType.add)
                rden = wp.tile([P, 2, H], BF, tag="rden")
                with nc.allow_low_precision(
                        reason="bf16 softmax normalization, 2^-8 rel err"):
                    nc.vector.reciprocal(rden[:], den[:])
                att = wp.tile([P, 2, H, H], BF, tag="att")
                nc.gpsimd.tensor_tensor(
                    out=att[:], in0=ex[:],
                    in1=rden[:].unsqueeze(3).to_broadcast([P, 2, H, H]),
                    op=mybir.AluOpType.mult)
                st["att", m] = att

            def S4b(m):     # DVE+Pool: p2 = att (x) vh (per half)
                qs_sb = st.pop(("qs_sb", m))
                att = st.pop(("att", m))
                p2 = pr.tile([P, 2, H, D, H], BF, tag="p2")
                gs = H - PG_P2
                for j in range(2):
                    vh = qs_sb[:, j, EMB:2 * EMB].rearrange(
                        "p (d g) -> p d g", d=D)
                    nc.vector.tensor_tensor(
                        out=p2[:, j, :, :, 0:gs],
                        in0=att[:, j, :, 0:gs].unsqueeze(2).to_broadcast(
                            [P, H, D, gs]),
                        in1=vh[:, :, 0:gs].unsqueeze(1).to_broadcast(
                            [P, H, D, gs]),
                        op=mybir.AluOpType.mult)
                    nc.gpsimd.tensor_tensor(
                        out=p2[:, j, :, :, gs:H],
                        in0=att[:, j, :, gs:H].unsqueeze(2).to_broadcast(
                            [P, H, D, PG_P2]),
                        in1=vh[:, :, gs:H].unsqueeze(1).to_broadcast(
                            [P, H, D, PG_P2]),
                        op=mybir.AluOpType.mult)
                st["p2", m] = p2

            def S5a(m):     # PE: pair transposing g-sum -> ov^T in PSUM
                p2 = st.pop(("p2", m))
                ps_ov = pov.tile([P, 2, 2, P], FP, space="PSUM", tag="ov")
                for j, b in enumerate(blocks_of(m)):
                    for hf in range(2):
                        for g in range(H):
                            lhsT = p2[:, j, 4 * hf:4 * hf + 4, :, g
                                      ].rearrange("p h d -> p (h d)")
                            nc.tensor.matmul(
                                out=ps_ov[:, j, hf, :], lhsT=lhsT,
                                rhs=ident[:],
                                start=(g == 0), stop=(g == H - 1),
                                skip_group_check=True)
                st["ps_ov", m] = ps_ov

            def S5b(m):     # ACT op3: ov^T pair -> SBUF bf16
                ps_ov = st.pop(("ps_ov", m))
                ovT = wp.tile([P, 2, 2, P], BF, tag="ovT")
                nc.scalar.copy(ovT[:], ps_ov[:])
                st["ovT", m] = ovT

            def S5c(m):     # PE: pair out^T = Wc^T @ ov^T (bf16)
                ovT = st.pop(("ovT", m))
                ps_o = po.tile([P, 2, 2, P], FP, space="PSUM", tag="o")
                for j, b in enumerate(blocks_of(m)):
                    for cf in range(2):
                        for hh in range(2):
                            nc.tensor.matmul(
                                out=ps_o[:, j, cf, :],
                                lhsT=wt["Wc"][:, hh, cf * P:(cf + 1) * P],
                                rhs=ovT[:, j, hh, :],
                                start=(hh == 0),
                                stop=(hh == 1) and not has_bc,
                                skip_group_check=True)
                        if has_bc:
                            nc.tensor.matmul(
                                out=ps_o[:, j, cf, :],
                                lhsT=bc_sb[:, cf * P:(cf + 1) * P],
                                rhs=ones1[:],
                                start=False, stop=True,
                                skip_group_check=True)
                st["ps_o", m] = ps_o

            def S5d(m):     # ACT op4: fp16 pair out; SP: DMA pair
                ps_o = st.pop(("ps_o", m))
                fo = ql.tile([P, 2, 2, P], F16, tag="fo")
                nc.scalar.copy(fo[:], ps_o[:])
                b = 2 * m
                hi = min(b + 2, NB)
                nc.sync.dma_start(outT_d[:, b:hi, :, :], fo[:, 0:hi - b, :, :])

            # list order = per-engine priority order; exp (S3f) last so its
            # wait on the same-iteration d-reduce doesn't delay the copies
            import os
            _p = os.environ.get("KPERM", "2")
            if _p == "0":
                stages = [(0, S0), (3, S2), (1, S1), (2, S1c), (3, S2c),
                          (4, S3b), (5, S3p), (6, S4r), (7, S4b), (8, S5a),
                          (9, S5b), (9, S5c), (10, S5d), (5, S3f)]
            elif _p == "1":   # copies earlier in priority
                stages = [(0, S0), (2, S1c), (3, S2c), (9, S5b), (3, S2),
                          (1, S1), (4, S3b), (5, S3p), (6, S4r), (7, S4b),
                          (8, S5a), (9, S5c), (10, S5d), (5, S3f)]
            elif _p == "2":   # tail stages high priority
                stages = [(0, S0), (9, S5b), (9, S5c), (10, S5d), (8, S5a),
                          (7, S4b), (6, S4r), (5, S3p), (4, S3b), (3, S2),
                          (3, S2c), (2, S1c), (1, S1), (5, S3f)]
            elif _p == "3":   # DVE/Pool work first
                stages = [(0, S0), (4, S3b), (7, S4b), (6, S4r), (3, S2),
                          (1, S1), (2, S1c), (3, S2c), (5, S3p), (8, S5a),
                          (9, S5b), (9, S5c), (10, S5d), (5, S3f)]
            DEPTH = 11
            import os as _os
            _pf = _os.environ.get("KPF", "1") == "1"
            NP2 = (NB + 1) // 2
            for i in range(NP2 + DEPTH - 1):
                for off, fn in stages:
                    if fn is S0 and _pf:
                        if i == 0:
                            fn(0)
                        if i + 1 < NP2:
                            fn(i + 1)
                        continue
                    mm = i - off
                    if 0 <= mm < NP2:
                        fn(mm)

    if split_waits:
        _split_sync_waits(nc)
    return nc


# --------------------------------------------------------------- host prep
def _prep(q, k, v, edge_index, Wq, bq, Wk, bk, Wv, bv, Wc, bc):
    A = np.asarray(edge_index[0], dtype=np.int64)
    B = np.asarray(edge_index[1], dtype=np.int64)
    order = np.argsort(A, kind="stable")
    A_s = A[order]
    B_s = B[order]

    core_lo = np.searchsorted(A_s, np.arange(NCORES) * NPC, side="left")
    core_hi = np.searchsorted(A_s, (np.arange(NCORES) + 1) * NPC, side="left")

    # --- per-core 2-deep slot assignment (vectorized)
    per_core = []
    npair = np.zeros((NCORES, NB), dtype=np.int64)
    for o in range(NCORES):
        a = A_s[core_lo[o]:core_hi[o]] - o * NPC      # local dest, ascending
        bi = B_s[core_lo[o]:core_hi[o]]
        n = len(a)
        first = np.searchsorted(a, a, side="left")
        rank = np.arange(n) - first
        depth = rank % 2
        cnt_d = np.bincount(a, minlength=NPC_PAD)
        s_d = (cnt_d + 1) // 2
        s_cum = np.cumsum(s_d) - s_d                   # global slot prefix
        blk_start = s_cum[np.arange(NB) * P]           # first slot of block
        slot_in_blk = (s_cum[a] - blk_start[a // P]) + rank // 2
        t = slot_in_blk // P
        p = slot_in_blk % P
        blk = a // P
        np.maximum.at(npair[o], blk, t + 1)
        per_core.append((a, bi, blk, t, p, depth))
    pairs_per_block = np.maximum(1, npair.max(axis=0)).astype(int)

    # --- per-(block,tile) destination windows, union across cores
    TMAX = int(pairs_per_block.max())
    lo = np.full((NB, TMAX), P, dtype=np.int64)
    hi = np.full((NB, TMAX), -1, dtype=np.int64)
    for o in range(NCORES):
        a, bi, blk, t, p, depth = per_core[o]
        dl = a - blk * P
        np.minimum.at(lo, (blk, t), dl)
        np.maximum.at(hi, (blk, t), dl)
    windows = []
    for bidx in range(NB):
        wb = []
        for t in range(int(pairs_per_block[bidx])):
            if hi[bidx, t] < 0:
                wb.append((0, 0))
                continue
            doff = int(lo[bidx, t])
            W = int(hi[bidx, t]) - doff + 1
            W = min((W + 3) // 4 * 4, P - doff)
            wb.append((doff, W))
        windows.append(wb)
    SW = [sum(w for _, w in wb) for wb in windows]

    KOW = [int(pairs_per_block[b]) * 2 * EMB + SW[b] for b in range(NB)]
    ko_off = np.zeros(NB + 1, dtype=np.int64)
    ko_off[1:] = np.cumsum(KOW)
    ke_base = ko_off[:NB]
    oh_base = ko_off[:NB] + pairs_per_block * 2 * EMB
    oh_col = np.zeros((NB, TMAX), dtype=np.int64)
    doffs = np.zeros((NB, TMAX), dtype=np.int64)
    for bidx in range(NB):
        acc = 0
        for t in range(int(pairs_per_block[bidx])):
            oh_col[bidx, t] = acc
            acc += windows[bidx][t][1]
            doffs[bidx, t] = windows[bidx][t][0]

    k8 = (np.asarray(k, np.float32) * SQ).astype(NP_F8)
    kos = []
    for o in range(NCORES):
        a, bi, blk, t, p, depth = per_core[o]
        ko = np.zeros((P, int(ko_off[-1])), dtype=NP_F8)
        cstart = ke_base[blk] + t * 2 * EMB + depth * EMB
        cidx = cstart[:, None] + np.arange(EMB)[None, :]
        ko[p[:, None], cidx] = k8[bi]
        m0 = depth == 0
        cols = (oh_base[blk[m0]] + oh_col[blk[m0], t[m0]]
                + (a[m0] - blk[m0] * P - doffs[blk[m0], t[m0]]))
        ko[p[m0], cols] = 1.0
        kos.append(ko)

    cnt_nodes = np.bincount(A, minlength=N_NODES).astype(np.float32)
    invc_full = 1.0 / np.maximum(cnt_nodes, 1.0)
    invcs, cnts = [], []
    for o in range(NCORES):
        s = np.ones(NPC_PAD, dtype=np.float32)
        s[:NPC] = invc_full[o * NPC:(o + 1) * NPC]
        invcs.append(np.ascontiguousarray(s.reshape(NB, P).T))
        c = np.zeros(NPC_PAD, dtype=np.float32)
        c[:NPC] = cnt_nodes[o * NPC:(o + 1) * NPC]
        cnts.append((c * SPROJ).reshape(1, NPC_PAD).astype(NP_BF))

    # q fp8; v as fp8 value + fp8 residual (both *SQ, exact power-of-2)
    q8 = (np.asarray(q, np.float32) * SQ).astype(NP_F8)
    vs = np.asarray(v, np.float32) * SQ
    v8 = vs.astype(NP_F8)
    r8 = (vs - v8.astype(np.float32)).astype(NP_F8)
    qvrs = []
    for o in range(NCORES):
        qvr = np.zeros((P, NB, 6, P), dtype=NP_F8)
        for j, src in ((0, q8), (2, v8), (4, r8)):
            sT = np.zeros((EMB, NPC_PAD), dtype=NP_F8)
            sT[:, :NPC] = src[o * NPC:(o + 1) * NPC].T
            qvr[:, :, j, :] = sT[0:P].reshape(P, NB, P)
            qvr[:, :, j + 1, :] = sT[P:EMB].reshape(P, NB, P)
        qvrs.append(qvr)

    # Wv column permutation: vh lands as [n, (d, g)]
    WvT = np.ascontiguousarray(np.asarray(Wv, np.float32).T)
    WvT_perm = WvT.reshape(EMB, H, D).transpose(0, 2, 1).reshape(EMB, EMB)
    bv_perm = np.asarray(bv, np.float32).reshape(H, D).T.reshape(-1)

    def pack2(WT):
        # [ch, out] -> [ch%128, ch//128, out]
        a = np.ascontiguousarray(np.asarray(WT, np.float32)).reshape(
            2, P, EMB)
        return np.ascontiguousarray(a.transpose(1, 0, 2))

    Wv_s = WvT_perm * SW_
    Wv8 = Wv_s.astype(NP_F8)
    Rv8 = (Wv_s - Wv8.astype(np.float32)).astype(NP_F8)

    bias_flags = (bool(np.any(np.asarray(bq))), bool(np.any(np.asarray(bk))),
                  bool(np.any(np.asarray(bv))), bool(np.any(np.asarray(bc))))
    has_bq, has_bk, has_bv, has_bc = bias_flags

    com = {
        "Wq8": pack2(np.asarray(Wq, np.float32).T * SW_).astype(NP_F8),
        "Wk8": pack2(np.asarray(Wk, np.float32).T * SW_).astype(NP_F8),
        "Wv8": pack2(Wv8.astype(np.float32)).astype(NP_F8),
        "Rv8": pack2(Rv8.astype(np.float32)).astype(NP_F8),
        "Wc16": pack2(np.asarray(Wc, np.float32).T).astype(NP_BF),
    }
    if has_bq or has_bv:
        bqv = np.concatenate([np.asarray(bq, np.float32),
                              bv_perm]) * SPROJ
        com["bqv"] = bqv.reshape(1, 2 * EMB).astype(NP_BF)
    if has_bk:
        com["bk"] = np.asarray(bk, np.float32).reshape(1, EMB).astype(NP_BF)
    if has_bc:
        com["bc"] = np.asarray(bc, np.float32).reshape(1, EMB).astype(NP_BF)

    in_maps = []
    for o in range(NCORES):
        m = dict(com)
        m["qvr"] = qvrs[o]
        m["ko"] = kos[o]
        m["invc"] = invcs[o]
        if has_bk:
            m["cnt"] = cnts[o]
        in_maps.append(m)
    return pairs_per_block.tolist(), windows, bias_flags, in_maps


_LAST = {}


def kernel(q, k, v, edge_index, Wq, bq, Wk, bk, Wv, bv, Wc, bc, latent=None,
           _want_results=False, _trace=False):
    pairs_per_block, windows, bias_flags, in_maps = _prep(
        q, k, v, edge_index, Wq, bq, Wk, bk, Wv, bv, Wc, bc)
    key = str((pairs_per_block, windows, bias_flags))
    if _LAST.get("key") != key:
        _LAST["nc"] = build_nc(pairs_per_block, windows, bias_flags)
        _LAST["key"] = key
    nc = _LAST["nc"]

    res = run_bass_kernel_spmd(nc, in_maps, core_ids=list(range(NCORES)),
                               trace=_trace)
    out = np.empty((N_NODES, EMB), dtype=np.float32)
    for o in range(NCORES):
        oT = res.results[o]["outT"].astype(np.float32)   # [P, NB, 2, P]
        full = np.empty((EMB, NPC_PAD), dtype=np.float32)
        full[0:P] = oT[:, :, 0, :].reshape(P, NPC_PAD)
        full[P:EMB] = oT[:, :, 1, :].reshape(P, NPC_PAD)
        out[o * NPC:(o + 1) * NPC] = full[:, :NPC].T
    if _want_results:
        return out, res
    return out
